# revision 11
# baseline (speedup 1.0000x reference)
"""Trainium2 Bass kernel for nn_EstimatorQNN.

Math reduction: the reference applies a batch-independent 2x2 unitary U
(built from the 4 weights) to |psi> = [cos(th/2), sin(th/2)] with
th = x0 + x1, then returns |amp0|^2 - |amp1|^2.  By unitarity this
collapses to

    out = A*cos(th) + D*sin(th) = R*sin(th + phi)

with A = 2|U00|^2 - 1, D = 2*Re(U00*conj(U01)), R = hypot(A, D),
phi = atan2(A, D).  A/D/R/phi are 4 scalars computed on host from the
weights; the device does the memory-bound elementwise part.

Device chain per element (HW Sin table is only valid on [-pi, pi], so
range-reduce with the fp32 magic-number round trick):
    th' = (x_even + phi) + x_odd              scalar_tensor_tensor   (DVE)
    m   = th'*(1/2pi) + MAGIC                 tensor_scalar          (DVE)
    k2  = (m - MAGIC)*2pi                     tensor_scalar          (DVE)
    psi = th' - k2                            tensor_tensor          (DVE)
    s   = Sin(psi)                            activation             (ACT)
    y   = s * R                               activation Copy        (ACT)

Raw-Bass hand-scheduled pipeline (no Tile framework): loads are enqueued
up-front on both HWDGE rings (SP + ACT), tile sizes ramp up so the first
compute starts early, and partition lines are 16 KiB for the big tiles
(full per-SDMA-engine rate).  Pure data parallel over 8 NeuronCores.
"""

import math
from contextlib import ExitStack

import numpy as np

B_FULL = 8388608
N_CORES = 8
B_SHARD = B_FULL // N_CORES  # 1048576

# per-tile input cols (per partition); sum * 128 == B_SHARD * 2
TILE_COLS = [1024, 1024, 2048, 4096, 4096, 4096]
assert sum(TILE_COLS) * 128 == B_SHARD * 2

MAGIC = 12582912.0                 # 1.5 * 2**23: fp32 round-to-nearest-int
TWO_PI = 6.283185307179586
INV_2PI = 1.0 / TWO_PI

# set by kernel() on each call; test.py reads it for profiling info
LAST_RESULT = None


def _host_constants(weights: np.ndarray):
    w = np.asarray(weights, dtype=np.float64)

    def rx(t):
        c, s = np.cos(t / 2), np.sin(t / 2)
        return np.array([[c, -1j * s], [-1j * s, c]], dtype=np.complex128)

    def rz(t):
        return np.array(
            [[np.exp(-1j * t / 2), 0], [0, np.exp(1j * t / 2)]], dtype=np.complex128
        )

    U = np.eye(2, dtype=np.complex128)
    for i in range(len(w) // 2):
        U = rz(w[2 * i + 1]) @ rx(w[2 * i]) @ U
    A = 2.0 * abs(U[0, 0]) ** 2 - 1.0
    D = 2.0 * (U[0, 0] * np.conj(U[0, 1])).real
    R = math.hypot(A, D)
    phi = math.atan2(A, D)
    return float(R), float(phi)


def _build_nc(R: float, phi: float):
    import concourse.bacc as bacc
    from concourse import mybir

    add = mybir.AluOpType.add
    sub = mybir.AluOpType.subtract
    mult = mybir.AluOpType.mult
    f32 = mybir.dt.float32
    Sin = mybir.ActivationFunctionType.Sin

    nc = bacc.Bacc(
        "TRN2",
        target_bir_lowering=False,
        debug=False,
        enable_asserts=False,
        num_devices=N_CORES,
    )
    x = nc.dram_tensor("x", [B_SHARD, 2], f32, kind="ExternalInput").ap()
    y = nc.dram_tensor("y", [B_SHARD, 1], f32, kind="ExternalOutput").ap()
    xf = x.rearrange("n t -> (n t)")
    yf = y.rearrange("n o -> (n o)")

    n_tiles = len(TILE_COLS)
    starts = [128 * sum(TILE_COLS[:i]) for i in range(n_tiles)]
    # DRAM-side per-tile views; each partition line is TILE_COLS[i]
    # contiguous floats (one DMA descriptor per line)
    xin = [
        xf[starts[i] : starts[i] + 128 * TILE_COLS[i]].rearrange(
            "(p f) -> p f", f=TILE_COLS[i]
        )
        for i in range(n_tiles)
    ]
    yout = [
        yf[starts[i] // 2 : starts[i] // 2 + 128 * (TILE_COLS[i] // 2)].rearrange(
            "(p h) -> p h", h=TILE_COLS[i] // 2
        )
        for i in range(n_tiles)
    ]

    HMAX = max(TILE_COLS) // 2

    # SBUF buffers: one input + one output slot per tile (no reuse, loads
    # all enqueued up-front), single slots for the DVE-serial temporaries,
    # double-buffered psi/s across the DVE->ACT boundary.
    t_bufs = [nc.alloc_sbuf_tensor(f"t{i}", [128, TILE_COLS[i]], f32) for i in range(n_tiles)]
    o_bufs = [nc.alloc_sbuf_tensor(f"o{i}", [128, TILE_COLS[i] // 2], f32) for i in range(n_tiles)]
    th = nc.alloc_sbuf_tensor("th", [128, HMAX], f32)
    mt = nc.alloc_sbuf_tensor("mt", [128, HMAX], f32)
    k2 = nc.alloc_sbuf_tensor("k2", [128, HMAX], f32)
    psi = [nc.alloc_sbuf_tensor(f"psi{j}", [128, HMAX], f32) for j in range(2)]
    sbuf = [nc.alloc_sbuf_tensor(f"s{j}", [128, HMAX], f32) for j in range(2)]

    with ExitStack() as ctx:
        lsem = [ctx.enter_context(nc.semaphore(f"l{i}")) for i in range(n_tiles)]
        osem = [ctx.enter_context(nc.semaphore(f"os{i}")) for i in range(n_tiles)]
        vs = ctx.enter_context(nc.semaphore("vs"))     # DVE psi_i ready (1/tile)
        vch = ctx.enter_context(nc.semaphore("vch"))   # DVE same-engine RAW chain (3/tile)
        ach = ctx.enter_context(nc.semaphore("ach"))   # ACT chain: sin=2i+1, mul=2i+2
        block = ctx.enter_context(nc.Block())

        @block.sync
        def _(sync):
            for i in range(0, n_tiles, 2):
                sync.dma_start(t_bufs[i].ap(), xin[i]).then_inc(lsem[i], 16)
            for i in range(0, n_tiles, 2):
                sync.wait_ge(ach, 2 * i + 2)
                sync.dma_start(yout[i], o_bufs[i].ap()).then_inc(osem[i], 16)
            for i in range(0, n_tiles, 2):
                sync.wait_ge(osem[i], 16)

        @block.vector
        def _(vector):
            for i in range(n_tiles):
                h = TILE_COLS[i] // 2
                vector.wait_ge(lsem[i], 16)
                if i >= 1:
                    # th free once TT_{i-1} (its reader) completed
                    vector.wait_ge(vs, i)
                t = t_bufs[i].ap()
                nc.vector.scalar_tensor_tensor(
                    th.ap()[:, :h], t[:, 0 : 2 * h : 2], phi, t[:, 1 : 2 * h : 2],
                    op0=add, op1=add,
                ).then_inc(vch, 1)
                vector.wait_ge(vch, 3 * i + 1)
                nc.vector.tensor_scalar(
                    mt.ap()[:, :h], th.ap()[:, :h], INV_2PI, MAGIC, op0=mult, op1=add
                ).then_inc(vch, 1)
                vector.wait_ge(vch, 3 * i + 2)
                nc.vector.tensor_scalar(
                    k2.ap()[:, :h], mt.ap()[:, :h], MAGIC, TWO_PI, op0=sub, op1=mult
                ).then_inc(vch, 1)
                vector.wait_ge(vch, 3 * i + 3)
                if i >= 2:
                    # psi[i%2] free once sin_{i-2} completed (ach = 2(i-2)+1)
                    vector.wait_ge(ach, 2 * i - 3)
                nc.vector.tensor_tensor(
                    psi[i % 2].ap()[:, :h], th.ap()[:, :h], k2.ap()[:, :h], op=sub
                ).then_inc(vs, 1)

        @block.scalar
        def _(scalar):
            for i in range(1, n_tiles, 2):
                scalar.dma_start(t_bufs[i].ap(), xin[i]).then_inc(lsem[i], 16)
            for i in range(n_tiles):
                h = TILE_COLS[i] // 2
                scalar.wait_ge(vs, i + 1)
                if i >= 2:
                    # s[i%2] free once mul_{i-2} (its reader) completed
                    scalar.wait_ge(ach, 2 * i - 2)
                nc.scalar.activation(
                    sbuf[i % 2].ap()[:, :h], psi[i % 2].ap()[:, :h], Sin,
                    bias=0.0, scale=1.0,
                ).then_inc(ach, 1)
                scalar.wait_ge(ach, 2 * i + 1)
                nc.scalar.mul(o_bufs[i].ap(), sbuf[i % 2].ap()[:, :h], R).then_inc(
                    ach, 1
                )
                if i % 2 == 1:
                    scalar.wait_ge(ach, 2 * i + 2)
                    scalar.dma_start(yout[i], o_bufs[i].ap()).then_inc(osem[i], 16)
            for i in range(1, n_tiles, 2):
                scalar.wait_ge(osem[i], 16)

    nc.compile()
    return nc


def kernel(inputs: np.ndarray, weights: np.ndarray, _trace: bool = False) -> np.ndarray:
    global LAST_RESULT
    from concourse.bass_utils import run_bass_kernel_spmd

    inputs = np.ascontiguousarray(np.asarray(inputs, dtype=np.float32))
    assert inputs.shape == (B_FULL, 2), inputs.shape

    R, phi = _host_constants(weights)
    nc = _build_nc(R, phi)

    in_maps = [
        {"x": inputs[c * B_SHARD : (c + 1) * B_SHARD]} for c in range(N_CORES)
    ]
    res = run_bass_kernel_spmd(
        nc, in_maps, core_ids=list(range(N_CORES)), trace=_trace
    )
    LAST_RESULT = res
    out = np.concatenate([r["y"] for r in res.results], axis=0)
    return out.astype(np.float32, copy=False)


# revision 13
# speedup vs baseline: 1.0401x; 1.0401x over previous
"""Trainium2 Bass kernel for nn_EstimatorQNN.

Math reduction: the reference applies a batch-independent 2x2 unitary U
(built from the 4 weights) to |psi> = [cos(th/2), sin(th/2)] with
th = x0 + x1, then returns |amp0|^2 - |amp1|^2.  By unitarity this
collapses to

    out = A*cos(th) + D*sin(th) = R*sin(th + phi)

with A = 2|U00|^2 - 1, D = 2*Re(U00*conj(U01)), R = hypot(A, D),
phi = atan2(A, D).  A/D/R/phi are 4 scalars computed on host from the
weights; the device does the memory-bound elementwise part.

Device chain per element (HW Sin table is only valid on [-pi, pi], so
range-reduce with the fp32 magic-number round trick):
    th' = (x_even + phi) + x_odd              scalar_tensor_tensor   (DVE)
    m   = th'*(1/2pi) + MAGIC                 tensor_scalar (DVE) or
                                              activation Identity (ACT)
    k2  = (m - MAGIC)*2pi                     tensor_scalar          (DVE)
    psi = th' - k2                            tensor_tensor          (DVE)
    s   = Sin(psi)                            activation             (ACT)
    y   = s * R                               activation Copy        (ACT)

Raw-Bass hand-scheduled pipeline (no Tile framework): loads enqueued
up-front on both HWDGE rings, tile sizes ramp up then down (fast fill,
short drain), 16 KiB partition lines on the big tiles for full per-SDMA
rate, the m-op of the big tiles offloaded to the Activation engine to
balance DVE.  A global op plan is linearized first and every RAW/WAR/WAW
hazard gets an explicit semaphore wait (TRN2 engine pipelines are deep;
even same-engine readers must sem-wait on the writer).  Pure data
parallel over 8 NeuronCores.
"""

import math
from contextlib import ExitStack

import numpy as np

B_FULL = 8388608
N_CORES = 8
B_SHARD = B_FULL // N_CORES  # 1048576

# per-tile input cols (per partition); sum * 128 == B_SHARD * 2
TILE_COLS = [1024, 2048, 4096, 4096, 2048, 2048, 1024]
assert sum(TILE_COLS) * 128 == B_SHARD * 2
M_ON_ACT = {2, 3}                  # tiles whose m-op runs on ACT

MAGIC = 12582912.0                 # 1.5 * 2**23: fp32 round-to-nearest-int
TWO_PI = 6.283185307179586
INV_2PI = 1.0 / TWO_PI

SYNC_TILES = (0, 2, 4, 6)          # loads+stores on the SP HWDGE ring
ACT_TILES = (1, 3, 5)              # loads+stores on the ACT HWDGE ring

LAST_RESULT = None


def _host_constants(weights: np.ndarray):
    w = np.asarray(weights, dtype=np.float64)

    def rx(t):
        c, s = np.cos(t / 2), np.sin(t / 2)
        return np.array([[c, -1j * s], [-1j * s, c]], dtype=np.complex128)

    def rz(t):
        return np.array(
            [[np.exp(-1j * t / 2), 0], [0, np.exp(1j * t / 2)]], dtype=np.complex128
        )

    U = np.eye(2, dtype=np.complex128)
    for i in range(len(w) // 2):
        U = rz(w[2 * i + 1]) @ rx(w[2 * i]) @ U
    A = 2.0 * abs(U[0, 0]) ** 2 - 1.0
    D = 2.0 * (U[0, 0] * np.conj(U[0, 1])).real
    R = math.hypot(A, D)
    phi = math.atan2(A, D)
    return float(R), float(phi)


def _plan_waits(plan):
    """Assign per-op semaphore waits for every RAW/WAR/WAW hazard.

    plan: list of dicts with keys eng, reads, writes, sem, inc.
    Returns waits per op as [(sem_key, value), ...], eliding waits already
    implied by an earlier wait on the same engine (sem values are monotone
    and each engine's stream order equals plan order restricted to it).
    """
    semval = {}
    writer = {}
    readers = {}
    seen = {}
    for op in plan:
        want = {}
        for b in op["reads"]:
            if b in writer:
                s, v = writer[b]
                want[s] = max(want.get(s, 0), v)
        for b in op["writes"]:
            for s, v in readers.get(b, []):
                want[s] = max(want.get(s, 0), v)
            if b in writer:
                s, v = writer[b]
                want[s] = max(want.get(s, 0), v)
        eng_seen = seen.setdefault(op["eng"], {})
        waits = []
        for s, v in want.items():
            if eng_seen.get(s, -1) < v:
                waits.append((s, v))
                eng_seen[s] = v
        op["waits"] = waits
        semval[op["sem"]] = semval.get(op["sem"], 0) + op["inc"]
        point = (op["sem"], semval[op["sem"]])
        for b in op["writes"]:
            writer[b] = point
            readers[b] = []
        for b in op["reads"]:
            readers.setdefault(b, []).append(point)
    return plan


def _build_nc(R: float, phi: float):
    import concourse.bacc as bacc
    from concourse import mybir

    add = mybir.AluOpType.add
    sub = mybir.AluOpType.subtract
    mult = mybir.AluOpType.mult
    f32 = mybir.dt.float32
    Sin = mybir.ActivationFunctionType.Sin
    Identity = mybir.ActivationFunctionType.Identity

    nc = bacc.Bacc(
        "TRN2",
        target_bir_lowering=False,
        debug=False,
        enable_asserts=False,
        num_devices=N_CORES,
    )
    x = nc.dram_tensor("x", [B_SHARD, 2], f32, kind="ExternalInput").ap()
    y = nc.dram_tensor("y", [B_SHARD, 1], f32, kind="ExternalOutput").ap()
    xf = x.rearrange("n t -> (n t)")
    yf = y.rearrange("n o -> (n o)")

    n_tiles = len(TILE_COLS)
    starts = [128 * sum(TILE_COLS[:i]) for i in range(n_tiles)]
    xin = [
        xf[starts[i] : starts[i] + 128 * TILE_COLS[i]].rearrange(
            "(p f) -> p f", f=TILE_COLS[i]
        )
        for i in range(n_tiles)
    ]
    yout = [
        yf[starts[i] // 2 : starts[i] // 2 + 128 * (TILE_COLS[i] // 2)].rearrange(
            "(p h) -> p h", h=TILE_COLS[i] // 2
        )
        for i in range(n_tiles)
    ]

    HMAX = max(TILE_COLS) // 2

    t_bufs = [nc.alloc_sbuf_tensor(f"t{i}", [128, TILE_COLS[i]], f32) for i in range(n_tiles)]
    o_bufs = [nc.alloc_sbuf_tensor(f"o{i}", [128, TILE_COLS[i] // 2], f32) for i in range(n_tiles)]
    th = [nc.alloc_sbuf_tensor(f"th{j}", [128, HMAX], f32) for j in range(2)]
    mt = [nc.alloc_sbuf_tensor(f"mt{j}", [128, HMAX], f32) for j in range(2)]
    k2 = [nc.alloc_sbuf_tensor(f"k2{j}", [128, HMAX], f32) for j in range(2)]
    psi = [nc.alloc_sbuf_tensor(f"psi{j}", [128, HMAX], f32) for j in range(2)]
    sb = [nc.alloc_sbuf_tensor(f"s{j}", [128, HMAX], f32) for j in range(2)]
    magic = nc.alloc_sbuf_tensor("magic", [128, 1], f32)

    def H(i):
        return TILE_COLS[i] // 2

    # ---- phase 1: global plan (linearized; every wait points backwards) --
    def op(eng, kind, i, reads, writes, sem, inc=1):
        return dict(eng=eng, kind=kind, i=i, reads=reads, writes=writes,
                    sem=sem, inc=inc)

    plan = []
    for i in SYNC_TILES:
        plan.append(op("s", "load", i, [], [f"t{i}"], f"l{i}", 16))
    for i in ACT_TILES:
        plan.append(op("a", "load", i, [], [f"t{i}"], f"l{i}", 16))
    plan.append(op("v", "memset", 0, [], ["magic"], "vq"))

    def dve_tile(i, with_m):
        plan.append(op("v", "stt", i, [f"t{i}"], [f"th{i % 2}"], "vq"))
        if with_m:
            plan.append(op("v", "m", i, [f"th{i % 2}"], [f"mt{i % 2}"], "vq"))

    def dve_tail(i):
        plan.append(op("v", "k2", i, [f"mt{i % 2}"], [f"k2{i % 2}"], "vq"))
        plan.append(op("v", "tt", i, [f"th{i % 2}", f"k2{i % 2}"],
                       [f"psi{i % 2}"], "vq"))

    def act_tile(i):
        plan.append(op("a", "sin", i, [f"psi{i % 2}"], [f"s{i % 2}"], "aq"))
        plan.append(op("a", "mul", i, [f"s{i % 2}"], [f"o{i}"], "aq"))
        ring = "s" if i in SYNC_TILES else "a"
        plan.append(op(ring, "store", i, [f"o{i}"], [], f"os{i}", 16))

    for i in (0, 1):
        dve_tile(i, with_m=True)
        dve_tail(i)
        act_tile(i)
    # tiles 2,3: th -> mt on ACT while DVE prefetches the next STT
    dve_tile(2, with_m=False)
    plan.append(op("a", "m", 2, [f"th{2 % 2}", "magic"], [f"mt{2 % 2}"], "aq"))
    dve_tile(3, with_m=False)
    plan.append(op("a", "m", 3, [f"th{3 % 2}", "magic"], [f"mt{3 % 2}"], "aq"))
    dve_tail(2)
    act_tile(2)
    dve_tail(3)
    act_tile(3)
    for i in (4, 5, 6):
        dve_tile(i, with_m=True)
        dve_tail(i)
        act_tile(i)

    _plan_waits(plan)

    # ---- phase 2: emit per-engine streams ---------------------------------
    with ExitStack() as ctx:
        sems = {}
        for i in range(n_tiles):
            sems[f"l{i}"] = ctx.enter_context(nc.semaphore(f"l{i}"))
            sems[f"os{i}"] = ctx.enter_context(nc.semaphore(f"os{i}"))
        sems["vq"] = ctx.enter_context(nc.semaphore("vq"))
        sems["aq"] = ctx.enter_context(nc.semaphore("aq"))
        block = ctx.enter_context(nc.Block())

        def emit(o, eng):
            for s, v in o["waits"]:
                eng.wait_ge(sems[s], v)
            i = o["i"]
            h = H(i)
            j = i % 2
            k = o["kind"]
            if k == "load":
                inst = eng.dma_start(t_bufs[i].ap(), xin[i])
            elif k == "store":
                inst = eng.dma_start(yout[i], o_bufs[i].ap())
            elif k == "memset":
                inst = nc.vector.memset(magic.ap(), MAGIC)
            elif k == "stt":
                t = t_bufs[i].ap()
                inst = nc.vector.scalar_tensor_tensor(
                    th[j].ap()[:, :h], t[:, 0 : 2 * h : 2], phi,
                    t[:, 1 : 2 * h : 2], op0=add, op1=add,
                )
            elif k == "m" and o["eng"] == "v":
                inst = nc.vector.tensor_scalar(
                    mt[j].ap()[:, :h], th[j].ap()[:, :h],
                    INV_2PI, MAGIC, op0=mult, op1=add,
                )
            elif k == "m":
                inst = nc.scalar.activation(
                    mt[j].ap()[:, :h], th[j].ap()[:, :h], Identity,
                    bias=magic.ap(), scale=INV_2PI,
                )
            elif k == "k2":
                inst = nc.vector.tensor_scalar(
                    k2[j].ap()[:, :h], mt[j].ap()[:, :h],
                    MAGIC, TWO_PI, op0=sub, op1=mult,
                )
            elif k == "tt":
                inst = nc.vector.tensor_tensor(
                    psi[j].ap()[:, :h], th[j].ap()[:, :h],
                    k2[j].ap()[:, :h], op=sub,
                )
            elif k == "sin":
                inst = nc.scalar.activation(
                    sb[j].ap()[:, :h], psi[j].ap()[:, :h], Sin,
                    bias=0.0, scale=1.0,
                )
            elif k == "mul":
                inst = nc.scalar.mul(o_bufs[i].ap(), sb[j].ap()[:, :h], R)
            else:
                raise AssertionError(k)
            inst.then_inc(sems[o["sem"]], o["inc"])

        @block.sync
        def _(sync):
            for o in plan:
                if o["eng"] == "s":
                    emit(o, sync)
            for i in SYNC_TILES:
                sync.wait_ge(sems[f"os{i}"], 16)

        @block.vector
        def _(vector):
            for o in plan:
                if o["eng"] == "v":
                    emit(o, vector)

        @block.scalar
        def _(scalar):
            for o in plan:
                if o["eng"] == "a":
                    emit(o, scalar)
            for i in ACT_TILES:
                scalar.wait_ge(sems[f"os{i}"], 16)

    nc.compile()
    return nc


def kernel(inputs: np.ndarray, weights: np.ndarray, _trace: bool = False) -> np.ndarray:
    global LAST_RESULT
    from concourse.bass_utils import run_bass_kernel_spmd

    inputs = np.ascontiguousarray(np.asarray(inputs, dtype=np.float32))
    assert inputs.shape == (B_FULL, 2), inputs.shape

    R, phi = _host_constants(weights)
    nc = _build_nc(R, phi)

    in_maps = [
        {"x": inputs[c * B_SHARD : (c + 1) * B_SHARD]} for c in range(N_CORES)
    ]
    res = run_bass_kernel_spmd(
        nc, in_maps, core_ids=list(range(N_CORES)), trace=_trace
    )
    LAST_RESULT = res
    out = np.concatenate([r["y"] for r in res.results], axis=0)
    return out.astype(np.float32, copy=False)
